# revision 9
# baseline (speedup 1.0000x reference)
"""Multi-head attention (B=8, S=1024, E=768, H=12) on 8 trn2 NeuronCores.

Strategy: batch-parallel — core b processes batch element b end-to-end, no
collectives.  Projections/attention/output in bf16 with fp32 PSUM; the
score matmul runs in fp8-e4m3 DoubleRow mode (2 contraction rows per
partition -> full 128-row array use at half the stream cycles) with a
q-side residual (q ~= q8 + qr8) so only the k-side fp8 quantization error
survives (~6e-3 end-to-end, vs 2.4e-3 all-bf16).

Per-core dataflow (token s/t, feature e, head h, head-dim d):
  xT[e, s]    PE-transpose of x in bf16 (cast on ACT first; 2 DMA waves)
  q/k proj    psum[hd, s-chunk] = Wq/Wk-tile^T @ xT; DVE writes
              qf[128, 2, 1024] fp8 (blk0 = fp8(psum+b), blk1 = residual),
              kf same with blk1 = copy of blk0 (SBUF DMA)
  v[t, hdA]   xT_aug^T @ WvT_aug  (ones row gives bias; col h*65+64 is
              all-ones -> softmax denominator), interleaved with pair 0
  scoresT     DoubleRow fp8: lhsT = kf[h*64:, :, t-tile], rhs = qf[h*64:,
              :, s-chunk]; 256 cycles per [128, 512] output
  expT        ACT exp with scale=1/8 (scores kept unscaled), bf16
  attn_aug    v_aug^T @ expT accumulated over t (row 64 = denominator)
  normalize   DVE reciprocal of den row; Pool partition_broadcast to 64
              rows; Pool/DVE multiply -> catT bf16
  out[s, f]   catT_aug^T @ WoT_aug (bias row folded), ACT copy, DMA out
"""

import os
import numpy as np
import ml_dtypes

B, S, E, H, DH = 8, 1024, 768, 12, 64
EA = E + 1          # augmented contraction dim (ones/bias row)
HW = DH + 1         # per-head V width (d cols + ones col)
VW = H * HW         # 780
NT = S // 128       # 8 token tiles
NE = E // 128       # 6 feature tiles

_cache = {}


def _split_multiwaits(nc):
    """This toolchain's walrus encodes at most one sync-wait per instruction
    (two for EventSemaphore).  Tile's epilogue can attach more; hoist the
    extras onto same-engine NOPs placed immediately before the instruction —
    the engine sequencer executes in order, so semantics are unchanged."""
    import concourse.mybir as mybir

    for bb in nc.main_func.blocks:
        out, changed = [], False
        for ins in bb.instructions:
            si = ins.sync_info
            cap = 2 if isinstance(ins, mybir.InstEventSemaphore) else 1
            if si is not None and si.on_wait and len(si.on_wait) > cap:
                waits = list(si.on_wait)
                for w_i, w in enumerate(waits[:-cap]):
                    out.append(mybir.InstNoOp(
                        name=f"{ins.name}-wsplit{w_i}",
                        engine=ins.engine,
                        sync_info=mybir.SyncInfo(on_wait=[w], on_update=[]),
                        bass_nofuse=True,
                    ))
                ins.sync_info = mybir.SyncInfo(
                    on_wait=waits[-cap:], on_update=list(si.on_update))
                changed = True
            out.append(ins)
        if changed:
            bb.instructions = out


def _dedupe_ldweights(nc):
    """Delete an InstLdweights when the immediately-preceding PE-stream
    instructions are its identical twin followed only by plain (non-transpose)
    matmuls — the weights are still resident in the array.  Only waitless,
    updateless LDWs are removed."""
    import concourse.mybir as mybir

    ndel = 0
    for bb in nc.main_func.blocks:
        out = []
        prev_key = None          # signature of weights currently in the array
        changed = False
        for ins in bb.instructions:
            if isinstance(ins, mybir.InstLdweights):
                si = ins.sync_info
                clean = not si or (not si.on_wait and not si.on_update)
                key = (str(ins.ins[0]), str(ins.tile_position),
                       str(ins.perf_mode), str(ins.is_transpose))
                if clean and key == prev_key:
                    ndel += 1
                    changed = True
                    continue
                prev_key = key
            elif isinstance(ins, mybir.InstMatmult):
                if ins.is_transpose:
                    prev_key = None   # transpose streams data into the array
            elif ins.engine == mybir.EngineType.PE:
                prev_key = None
            out.append(ins)
        if changed:
            bb.instructions = out
    return ndel


def _build_bass(split_waits=True):
    import concourse.bass as bass
    import concourse.tile as tile
    import concourse.mybir as mybir
    from concourse.masks import make_identity
    from contextlib import ExitStack

    f32 = mybir.dt.float32
    bf16 = mybir.dt.bfloat16
    fp8 = mybir.dt.float8e4
    EXP = mybir.ActivationFunctionType.Exp
    DR = mybir.MatmulPerfMode.DoubleRow
    ADD = mybir.AluOpType.add
    SUB = mybir.AluOpType.subtract

    nc = bass.Bass(trn_type="TRN2")

    x_d = nc.dram_tensor("x", [S, E], f32, kind="ExternalInput")
    wqt_d = nc.dram_tensor("wqt", [E, E], bf16, kind="ExternalInput")
    wkt_d = nc.dram_tensor("wkt", [E, E], bf16, kind="ExternalInput")
    bq_d = nc.dram_tensor("bq", [E, 1], f32, kind="ExternalInput")
    bk_d = nc.dram_tensor("bk", [E, 1], f32, kind="ExternalInput")
    wvt_d = nc.dram_tensor("wvt", [EA, VW], bf16, kind="ExternalInput")
    wot_d = nc.dram_tensor("wot", [EA, E], bf16, kind="ExternalInput")
    out_d = nc.dram_tensor("out", [S, E], f32, kind="ExternalOutput")

    with tile.TileContext(nc) as tc, ExitStack() as ctx:
        singles = ctx.enter_context(tc.tile_pool(name="singles", bufs=1))

        ident = singles.tile([128, 128], bf16)
        make_identity(nc, ident)

        ones_row = singles.tile([1, 1024], bf16)
        nc.vector.memset(ones_row, 1.0)

        # ---- input DMAs (issue order == transfer order on the SP queue) ----
        xsb = singles.tile([128, NT * E], f32, tag="x", name="xall")
        for c in range(4):
            x_src = bass.AP(tensor=x_d, offset=c * 2 * 128 * E,
                            ap=[[E, 128], [128 * E, 2], [1, E]])
            nc.sync.dma_start(out=xsb[:, c * 2 * E:(c + 1) * 2 * E], in_=x_src)

        bqs, bks = [], []
        for m in range(NE):
            t = singles.tile([128, 1], f32, tag=f"bq{m}", name=f"bq{m}")
            nc.sync.dma_start(out=t, in_=bq_d[m * 128:(m + 1) * 128, :])
            bqs.append(t)
            t = singles.tile([128, 1], f32, tag=f"bk{m}", name=f"bk{m}")
            nc.sync.dma_start(out=t, in_=bk_d[m * 128:(m + 1) * 128, :])
            bks.append(t)

        class WView:
            """All k-tiles of a weight in one SBUF tile (one DMA)."""
            def __init__(self, all_tile, width, bias_tile):
                self.all, self.width, self.bias = all_tile, width, bias_tile

            def __getitem__(self, k):
                if self.bias is not None and k == NE:
                    return self.bias
                return _WSlice(self, k)

        class _WSlice:
            def __init__(self, v, k):
                self.v, self.k = v, k

            def __getitem__(self, idx):
                _, cols = idx
                off = self.k * self.v.width
                return self.v.all[:, off + cols.start:off + cols.stop]

        def load_w(dram, width, rows):
            t = singles.tile([128, NE * width], bf16, tag=f"w{dram.name}",
                             name=f"w{dram.name}")
            w_src = bass.AP(tensor=dram, offset=0,
                            ap=[[width, 128], [128 * width, NE], [1, width]])
            nc.sync.dma_start(out=t, in_=w_src)
            bias_t = None
            if rows % 128:
                bias_t = singles.tile([1, width], bf16, tag=f"w{dram.name}b",
                                      name=f"w{dram.name}b")
                nc.sync.dma_start(out=bias_t, in_=dram[E:EA, :])
            return WView(t, width, bias_t)

        wq = load_w(wqt_d, E, E)
        wk = load_w(wkt_d, E, E)
        wv = load_w(wvt_d, VW, EA)
        wo = load_w(wot_d, E, EA)

        # ---- P1: x -> xT (bf16 transposes; ACT does the f32->bf16 cast) ----
        xbf = singles.tile([128, NT * E], bf16, tag="xbf", name="xbf")
        xt = [singles.tile([128, S], bf16, tag=f"xt{j}", name=f"xt{j}")
              for j in range(NE)]

        with tc.tile_pool(name="ps_xt", bufs=4, space="PSUM") as ps_xt:
            for w in range(2):          # two waves of 4 token tiles
                nc.scalar.copy(xbf[:, w * 4 * E:(w + 1) * 4 * E],
                               xsb[:, w * 4 * E:(w + 1) * 4 * E])
                for j in range(NE):
                    ps = ps_xt.tile([128, 512], bf16, tag="pxt",
                                    name=f"pxt{w}_{j}")
                    for ii in range(4):
                        i = w * 4 + ii
                        nc.tensor.transpose(
                            ps[:, ii * 128:(ii + 1) * 128],
                            xbf[:, i * E + j * 128:i * E + (j + 1) * 128],
                            ident,
                        )
                    nc.vector.tensor_copy(
                        xt[j][:, w * 512:(w + 1) * 512], ps)

        def xa(k):  # augmented xT rows
            return xt[k] if k < NE else ones_row

        # ---- steady-state tiles ----
        vt = [singles.tile([128, VW], bf16, tag=f"vt{i}", name=f"vt{i}")
              for i in range(NT)]
        catt = [singles.tile([128, S], bf16, tag=f"ct{j}", name=f"ct{j}")
                for j in range(NE)]

        with ExitStack() as sctx:
            qk8p = sctx.enter_context(tc.tile_pool(name="qk8", bufs=2))
            expp = sctx.enter_context(tc.tile_pool(name="exp", bufs=18))
            normp = sctx.enter_context(tc.tile_pool(name="norm", bufs=2))
            ps_proj = sctx.enter_context(
                tc.tile_pool(name="ps_proj", bufs=2, space="PSUM"))
            ps_sc = sctx.enter_context(
                tc.tile_pool(name="ps_sc", bufs=2, space="PSUM"))
            dscr = sctx.enter_context(
                tc.tile_pool(name="dscr", bufs=4, space="DRAM"))

            qfs, kfs = {}, {}

            def emit_qk(hp):
                qf = qk8p.tile([128, 2, S], fp8, tag="qf", name=f"qf{hp}")
                kf = qk8p.tile([128, 2, S], fp8, tag="kf", name=f"kf{hp}")
                qfs[hp], kfs[hp] = qf, kf
                for dst, w, b, is_q in ((kf, wk, bks, False),
                                        (qf, wq, bqs, True)):
                    for sc in range(2):
                        sl = slice(sc * 512, (sc + 1) * 512)
                        ps = ps_proj.tile([128, 512], f32, tag="pp",
                                          name=f"pp{hp}_{int(is_q)}{sc}")
                        for k in range(NE):
                            nc.tensor.matmul(
                                ps,
                                lhsT=w[k][:, hp * 128:(hp + 1) * 128],
                                rhs=xt[k][:, sl],
                                start=(k == 0), stop=(k == NE - 1),
                            )
                        nc.vector.tensor_scalar_add(
                            dst[:, 0:1, sl], ps, b[hp])
                        if is_q:
                            nc.vector.scalar_tensor_tensor(
                                dst[:, 1:2, sl], ps, b[hp], dst[:, 0:1, sl],
                                ADD, SUB)
                nc.sync.dma_start(out=kf[:, 1:2, :], in_=kf[:, 0:1, :])

            def emit_v():
                with tc.tile_pool(name="ps_v", bufs=1, space="PSUM") as ps_v:
                    for i in range(NT):
                        ps = ps_v.tile([128, VW], f32, tag="pv", name=f"pv{i}")
                        for k in range(NE + 1):
                            for off, sz in ((0, 512), (512, VW - 512)):
                                nc.tensor.matmul(
                                    ps[:, off:off + sz],
                                    lhsT=xa(k)[:, i * 128:(i + 1) * 128],
                                    rhs=wv[k][:, off:off + sz],
                                    start=(k == 0), stop=(k == NE),
                                )
                        nc.vector.tensor_copy(vt[i], ps)

            emit_qk(0)

            for hp in range(H // 2):
                qf, kf = qfs.pop(hp), kfs.pop(hp)
                exps = [[], []]
                for t in range(NT):
                    for half in range(2):
                        hb = half * 64
                        ps = ps_sc.tile([128, 1024], f32, tag="sc",
                                        name=f"sc{hp}_{t}_{half}")
                        for sc in range(2):
                            nc.tensor.matmul(
                                ps[:, sc * 512:(sc + 1) * 512],
                                lhsT=kf[hb:hb + 64, :,
                                        t * 128:(t + 1) * 128],
                                rhs=qf[hb:hb + 64, :,
                                       sc * 512:(sc + 1) * 512],
                                start=True, stop=True, perf_mode=DR,
                            )
                        ex = expp.tile([128, 1024], bf16, tag="e",
                                       name=f"e{hp}_{t}_{half}")
                        nc.scalar.activation(ex, ps, EXP, scale=0.125)
                        exps[half].append(ex)
                if hp == 0:
                    emit_v()
                    # V psum banks freed; attention psum takes their place.
                    ps_at = sctx.enter_context(
                        tc.tile_pool(name="ps_at", bufs=1, space="PSUM"))
                if hp + 1 < H // 2:
                    emit_qk(hp + 1)
                for half in range(2):
                    head = hp * 2 + half
                    pa = ps_at.tile([HW, 1024], f32, tag="at",
                                    name=f"at{head}")
                    for t in range(NT):
                        for sc in range(2):
                            nc.tensor.matmul(
                                pa[:, sc * 512:(sc + 1) * 512],
                                lhsT=vt[t][:, head * HW:(head + 1) * HW],
                                rhs=exps[half][t][:, sc * 512:(sc + 1) * 512],
                                start=(t == 0), stop=(t == NT - 1),
                            )
                    asb = normp.tile([HW, 1024], f32, tag="asb",
                                     name=f"asb{head}")
                    nc.vector.tensor_copy(asb, pa)
                    rcp = normp.tile([1, 1024], f32, tag="rcp",
                                     name=f"rcp{head}")
                    nc.vector.reciprocal(rcp, asb[64:65, :])
                    dn = dscr.tile([1, 1024], f32, tag="dn", name=f"dn{head}")
                    nc.gpsimd.dma_start(out=dn, in_=rcp)
                    rcb = normp.tile([64, 1024], f32, tag="rcb",
                                     name=f"rcb{head}")
                    nc.gpsimd.dma_start(
                        out=rcb, in_=dn[0].partition_broadcast(64))
                    muleng = nc.vector if hp == H // 2 - 1 else nc.gpsimd
                    muleng.tensor_mul(
                        catt[hp][half * 64:(half + 1) * 64, :],
                        asb[0:64, :], rcb)

        # ---- P4: output projection ----
        def ca(k):
            return catt[k] if k < NE else ones_row

        with tc.tile_pool(name="osb", bufs=3) as osb, \
             tc.tile_pool(name="ps_o", bufs=2, space="PSUM") as ps_o:
            KL = [0, 1, 2, 3, 4, NE, 5]
            for m in range(NT):
                ps = ps_o.tile([128, E], f32, tag="po", name=f"po{m}")
                for k in KL:
                    for off, sz in ((0, 512), (512, E - 512)):
                        nc.tensor.matmul(
                            ps[:, off:off + sz],
                            lhsT=ca(k)[:, m * 128:(m + 1) * 128],
                            rhs=wo[k][:, off:off + sz],
                            start=(k == KL[0]), stop=(k == KL[-1]),
                        )
                ot = osb.tile([128, E], f32, tag="o", name=f"ot{m}")
                nc.scalar.copy(ot, ps)
                nc.sync.dma_start(out=out_d[m * 128:(m + 1) * 128, :], in_=ot)

    _dedupe_ldweights(nc)
    if split_waits:
        _split_multiwaits(nc)
    return nc


def _prep_weights(Wq, bq, Wk, bk, Wv, bv, Wo, bo):
    bf16 = ml_dtypes.bfloat16

    wqt = np.asarray(Wq, np.float32).reshape(H * DH, E).T.astype(bf16)
    wkt = np.asarray(Wk, np.float32).reshape(H * DH, E).T.astype(bf16)
    bqv = np.asarray(bq, np.float32).reshape(E, 1).astype(np.float32)
    bkv = np.asarray(bk, np.float32).reshape(E, 1).astype(np.float32)

    wvt = np.zeros((EA, VW), np.float32)
    Wv = np.asarray(Wv, np.float32)
    bv = np.asarray(bv, np.float32)
    for h in range(H):
        wvt[0:E, h * HW:h * HW + DH] = Wv[h].T
        wvt[E, h * HW:h * HW + DH] = bv[h]
        wvt[E, h * HW + DH] = 1.0
    wvt = wvt.astype(bf16)

    Wo = np.asarray(Wo, np.float32)
    bo = np.asarray(bo, np.float32)
    wot = np.concatenate([Wo.T, bo.reshape(1, E)], axis=0).astype(bf16)
    return wqt, wkt, bqv, bkv, wvt, wot


def _install_ntff_shim():
    """Provide antenv.axon_hooks (absent in this image) so trace=True can
    drive NRT profiling through libaxon_pjrt.so.  Dev-only; harmless no-op
    when anything is missing."""
    import sys, types
    try:
        import antenv.axon_hooks  # noqa
        return
    except ImportError:
        pass
    try:
        import antenv
        mod = types.ModuleType("antenv.axon_hooks")
        _state = {}
        mod.set_axon_ntff_profile_hook = lambda h: _state.update(h=h)
        mod.get_axon_ntff_profile_hook = lambda: _state.get("h")
        sys.modules["antenv.axon_hooks"] = mod
        antenv.axon_hooks = mod
        from trn_agent_boot.trn_boot import _ntff_profile_via_ctypes
        hook = _ntff_profile_via_ctypes("/opt/axon/libaxon_pjrt.so")
        if hook is not None:
            mod.set_axon_ntff_profile_hook(hook)
    except Exception as e:  # pragma: no cover
        print(f"ntff shim failed: {e}")


def kernel(x, Wq, bq, Wk, bk, Wv, bv, Wo, bo):
    from concourse.bass_utils import run_bass_kernel_spmd

    if "nc" not in _cache:
        _cache["nc"] = _build_bass()
    nc = _cache["nc"]

    wqt, wkt, bqv, bkv, wvt, wot = _prep_weights(Wq, bq, Wk, bk, Wv, bv, Wo, bo)
    x = np.asarray(x, np.float32)
    in_maps = [
        {"x": np.ascontiguousarray(x[b]),
         "wqt": wqt, "wkt": wkt, "bq": bqv, "bk": bkv,
         "wvt": wvt, "wot": wot}
        for b in range(B)
    ]
    trace = bool(int(os.environ.get("MHA_TRACE", "0")))
    if trace:
        _install_ntff_shim()
    res = run_bass_kernel_spmd(nc, in_maps, list(range(B)), trace=trace)
    _cache["last_results"] = res
    return np.stack([res.results[b]["out"] for b in range(B)]).astype(np.float32)
